# revision 10
# baseline (speedup 1.0000x reference)
"""Trainium2 Bass kernel for multi-head attention (B=4, T=2048, C=1024, H=16).

Sharding: 8 cores = (batch b in 0..3) x (head-group g in 0..1, 8 heads each).
Per core: QKV projections for its 512 dims, attention for 8 heads, partial
output projection. Host sums the two per-batch partials and adds the biases
that fold out of the device computation:
  - bk drops entirely (softmax is invariant to per-query additive constants)
  - bv folds to host:   out += Wo @ bv   (softmax rows sum to 1)
  - bo added on host
  - bq is applied on-device in the Q-projection drain (scaled by 1/sqrt(dh))

Device layout (per core):
  QT, KT [512, 2048] fp32r  (head-dim on partitions -> scores without transpose)
  V      [2048, 512] bf16   (keys on partitions -> P@V without transpose)
  scores S^T [keys, queries] in PSUM -> exp on ACT -> E bf16 in SBUF
  denominators via ones-matmul on PE (partition-dim reduction)
  P@V accumulated over key chunks; normalize by 1/d on the DVE drain
  out-projection accumulated over the 4 head-pair chunks -> DMA to DRAM
"""
import os
import numpy as np

import concourse.bass as bass
import concourse.mybir as mybir
import concourse.tile as tile
from concourse import bacc

F32 = mybir.dt.float32
F32R = mybir.dt.float32r
BF16 = mybir.dt.bfloat16
AF = mybir.ActivationFunctionType

B, T, C = 4, 2048, 1024
H, CH = 16, 64
G = 512            # dims per head-group (8 heads)
NCIN = 8           # 128-chunks of C
NCOUT = 4          # 128-chunks of G
NTB = 4            # 512-wide t blocks
NKC = 16           # 128-wide key chunks
NQB = 4            # 512-wide query blocks
SCALE = 1.0 / np.sqrt(CH)


def build_nc(debug=False):
    nc = bacc.Bacc()
    xq = nc.declare_dram_parameter("xq", [C, T], F32R, isOutput=False)
    xk = nc.declare_dram_parameter("xk", [C, T], F32R, isOutput=False)
    xv = nc.declare_dram_parameter("xv", [C, T], F32R, isOutput=False)
    wq = nc.declare_dram_parameter("wq", [C, G], F32R, isOutput=False)
    wk = nc.declare_dram_parameter("wk", [C, G], F32R, isOutput=False)
    wv = nc.declare_dram_parameter("wv", [C, G], F32R, isOutput=False)
    wo = nc.declare_dram_parameter("wo", [G, C], F32R, isOutput=False)
    bq = nc.declare_dram_parameter("bq", [128, NCOUT], F32, isOutput=False)
    out = nc.declare_dram_parameter("out", [T, C], F32, isOutput=True)
    if debug:
        qt_d = nc.declare_dram_parameter("qt_d", [NCOUT, 128, T], F32R, isOutput=True)
        kt_d = nc.declare_dram_parameter("kt_d", [NCOUT, 128, T], F32R, isOutput=True)
        v_d = nc.declare_dram_parameter("v_d", [128, NKC, G], BF16, isOutput=True)
        e_d = nc.declare_dram_parameter("e_d", [2, 128, NKC, 512], BF16, isOutput=True)
        di_d = nc.declare_dram_parameter("di_d", [128, 512], F32, isOutput=True)
        ot_d = nc.declare_dram_parameter("ot_d", [128, NCOUT, 512], F32R, isOutput=True)

    xq_r = xq.rearrange("(c p) t -> p c t", p=128)
    xk_r = xk.rearrange("(c p) t -> p c t", p=128)
    xv_r = xv.rearrange("(c p) t -> p c t", p=128)

    with tile.TileContext(nc) as tc:
        with tc.tile_pool(name="persist", bufs=1) as persist:
            qt = [persist.tile([128, T], F32R, tag=f"qt{i}", name=f"qt{i}") for i in range(NCOUT)]
            kt = [persist.tile([128, T], F32R, tag=f"kt{i}", name=f"kt{i}") for i in range(NCOUT)]
            v_bf = persist.tile([128, NKC, G], BF16, tag="vbf")
            ones = persist.tile([128, 1], BF16, tag="ones")
            nc.vector.memset(ones, 1.0)

            # ---------------- Phase A: projections ----------------
            with tc.tile_pool(name="wA", bufs=1) as wA, \
                 tc.tile_pool(name="xs", bufs=2) as xs, \
                 tc.tile_pool(name="psA", bufs=4, space="PSUM") as psA:
                wq_sb = wA.tile([128, NCIN, G], F32R, tag="wq")
                wk_sb = wA.tile([128, NCIN, G], F32R, tag="wk")
                wv_sb = wA.tile([128, NCIN, G], F32R, tag="wv")
                bq_sb = wA.tile([128, NCOUT], F32, tag="bq")
                nc.default_dma_engine.dma_start(
                    out=wq_sb, in_=wq.rearrange("(c p) g -> p c g", p=128))
                nc.default_dma_engine.dma_start(
                    out=wk_sb, in_=wk.rearrange("(c p) g -> p c g", p=128))
                nc.default_dma_engine.dma_start(
                    out=wv_sb, in_=wv.rearrange("(c p) g -> p c g", p=128))
                nc.default_dma_engine.dma_start(out=bq_sb, in_=bq[:, :])

                # K projection (plain), then Q (bias+scale), then V (bf16)
                for tb in range(NTB):
                    xk_t = xs.tile([128, NCIN, 512], F32R, tag="xstream")
                    nc.default_dma_engine.dma_start(
                        out=xk_t, in_=xk_r[:, :, tb * 512:(tb + 1) * 512])
                    for co in range(NCOUT):
                        ps = psA.tile([128, 512], F32, tag="psA")
                        for ci in range(NCIN):
                            nc.tensor.matmul(
                                ps,
                                wk_sb[:, ci, co * 128:(co + 1) * 128],
                                xk_t[:, ci, :],
                                start=(ci == 0), stop=(ci == NCIN - 1))
                        nc.vector.tensor_copy(
                            out=kt[co][:, tb * 512:(tb + 1) * 512], in_=ps)
                for tb in range(NTB):
                    xq_t = xs.tile([128, NCIN, 512], F32R, tag="xstream")
                    nc.default_dma_engine.dma_start(
                        out=xq_t, in_=xq_r[:, :, tb * 512:(tb + 1) * 512])
                    for co in range(NCOUT):
                        ps = psA.tile([128, 512], F32, tag="psA")
                        for ci in range(NCIN):
                            nc.tensor.matmul(
                                ps,
                                wq_sb[:, ci, co * 128:(co + 1) * 128],
                                xq_t[:, ci, :],
                                start=(ci == 0), stop=(ci == NCIN - 1))
                        nc.scalar.activation(
                            qt[co][:, tb * 512:(tb + 1) * 512], ps,
                            AF.Identity,
                            bias=bq_sb[:, co:co + 1], scale=float(SCALE))
                for tb in range(NTB):
                    xv_t = xs.tile([128, NCIN, 512], F32R, tag="xstream")
                    nc.default_dma_engine.dma_start(
                        out=xv_t, in_=xv_r[:, :, tb * 512:(tb + 1) * 512])
                    for sub in range(4):
                        tcix = tb * 4 + sub
                        ps = psA.tile([128, 512], F32, tag="psA")
                        for ci in range(NCIN):
                            nc.tensor.matmul(
                                ps,
                                xv_t[:, ci, sub * 128:(sub + 1) * 128],
                                wv_sb[:, ci, :],
                                start=(ci == 0), stop=(ci == NCIN - 1))
                        nc.vector.tensor_copy(out=v_bf[:, tcix, :], in_=ps)
                if debug:
                    for co in range(NCOUT):
                        nc.default_dma_engine.dma_start(out=qt_d[co], in_=qt[co][:, :])
                        nc.default_dma_engine.dma_start(out=kt_d[co], in_=kt[co][:, :])
                    nc.default_dma_engine.dma_start(out=v_d[:, :, :], in_=v_bf[:, :, :])

            # ---------------- Phase B: attention + out-proj ----------------
            with tc.tile_pool(name="wB", bufs=1) as wB, \
                 tc.tile_pool(name="eb", bufs=2) as eb, \
                 tc.tile_pool(name="otp", bufs=2) as otp, \
                 tc.tile_pool(name="dv", bufs=2) as dv, \
                 tc.tile_pool(name="scp", bufs=2, space="PSUM") as scp, \
                 tc.tile_pool(name="pvp", bufs=2, space="PSUM") as pvp, \
                 tc.tile_pool(name="dpp", bufs=1, space="PSUM") as dpp, \
                 tc.tile_pool(name="pjp", bufs=1, space="PSUM") as pjp:
                wo_sb = wB.tile([128, NCOUT, C], F32R, tag="wo")
                nc.default_dma_engine.dma_start(
                    out=wo_sb, in_=wo.rearrange("(c p) g -> p c g", p=128))

                for qb in range(NQB):
                    ot_t = otp.tile([128, NCOUT, 512], F32R, tag="ot")
                    for p in range(NCOUT):
                        e0 = eb.tile([128, NKC, 512], BF16, tag="e0")
                        e1 = eb.tile([128, NKC, 512], BF16, tag="e1")
                        for h, e in ((0, e0), (1, e1)):
                            hs = h * 64
                            for grp in range(NKC // 2):
                                psc = scp.tile([128, 2, 512], F32, tag="sc")
                                for j in range(2):
                                    kc = grp * 2 + j
                                    nc.tensor.matmul(
                                        psc[:, j, :],
                                        kt[p][hs:hs + 64, kc * 128:(kc + 1) * 128],
                                        qt[p][hs:hs + 64, qb * 512:(qb + 1) * 512],
                                        start=True, stop=True)
                                nc.scalar.activation(
                                    e[:, grp * 2:grp * 2 + 2, :], psc, AF.Exp)
                        # denominators: ones.T @ E accumulated over key chunks
                        dps = dpp.tile([128, 512], F32, tag="dps")
                        for kc in range(NKC):
                            nc.tensor.matmul(dps[0:1, :], ones[:, 0:1],
                                             e0[:, kc, :],
                                             start=(kc == 0), stop=(kc == NKC - 1))
                            nc.tensor.matmul(dps[32:33, :], ones[:, 0:1],
                                             e1[:, kc, :],
                                             start=(kc == 0), stop=(kc == NKC - 1))
                        rec_lo = dv.tile([1, 512], F32, tag="rec_lo")
                        rec_hi = dv.tile([1, 512], F32, tag="rec_hi")
                        nc.vector.reciprocal(rec_lo[0:1, :], dps[0:1, :])
                        nc.vector.reciprocal(rec_hi[0:1, :], dps[32:33, :])
                        dbc_lo = dv.tile([64, 512], F32, tag="dbc_lo")
                        dbc_hi = dv.tile([64, 512], F32, tag="dbc_hi")
                        nc.gpsimd.partition_broadcast(dbc_lo[:, :], rec_lo[0:1, :],
                                                      channels=64)
                        nc.gpsimd.partition_broadcast(dbc_hi[:, :], rec_hi[0:1, :],
                                                      channels=64)
                        # P @ V (two heads col-packed)
                        pv = pvp.tile([128, 512], F32, tag="pv")
                        for kc in range(NKC):
                            nc.tensor.matmul(
                                pv[0:64, :],
                                v_bf[:, kc, p * 128:p * 128 + 64],
                                e0[:, kc, :],
                                start=(kc == 0), stop=(kc == NKC - 1))
                            nc.tensor.matmul(
                                pv[64:128, :],
                                v_bf[:, kc, p * 128 + 64:p * 128 + 128],
                                e1[:, kc, :],
                                start=(kc == 0), stop=(kc == NKC - 1))
                        nc.vector.tensor_mul(ot_t[0:64, p, :], pv[0:64, :],
                                             dbc_lo[:, :])
                        nc.vector.tensor_mul(ot_t[64:128, p, :], pv[64:128, :],
                                             dbc_hi[:, :])
                        if debug and qb == 0 and p == 0:
                            nc.default_dma_engine.dma_start(out=e_d[0], in_=e0[:, :, :])
                            nc.default_dma_engine.dma_start(out=e_d[1], in_=e1[:, :, :])
                            nc.default_dma_engine.dma_start(out=di_d[0:64, :], in_=dbc_lo[:, :])
                            nc.default_dma_engine.dma_start(out=di_d[64:128, :], in_=dbc_hi[:, :])
                    if debug and qb == 0:
                        nc.default_dma_engine.dma_start(out=ot_d[:, :, :], in_=ot_t[:, :, :])
                    # output projection for this query block
                    for tcx in range(4):
                        for n in range(2):
                            pj = pjp.tile([128, 512], F32, tag="pj")
                            for p in range(NCOUT):
                                nc.tensor.matmul(
                                    pj,
                                    ot_t[:, p, tcx * 128:(tcx + 1) * 128],
                                    wo_sb[:, p, n * 512:(n + 1) * 512],
                                    start=(p == 0), stop=(p == NCOUT - 1))
                            oj = dv.tile([128, 512], F32, tag="oj")
                            nc.vector.tensor_copy(out=oj, in_=pj)
                            r0 = qb * 512 + tcx * 128
                            nc.default_dma_engine.dma_start(
                                out=out[r0:r0 + 128, n * 512:(n + 1) * 512],
                                in_=oj)
    nc.finalize()
    return nc


_CACHE = {}


def _get_runner():
    """Compile once per process; return f(in_maps) -> list of out dicts."""
    if "runner" in _CACHE:
        return _CACHE["runner"]
    import jax
    from jax.sharding import Mesh, PartitionSpec
    from jax.experimental.shard_map import shard_map
    from concourse import bass2jax

    nc = build_nc()
    bass2jax.install_neuronx_cc_hook()
    in_names, out_names, out_avals, zero_shapes = [], [], [], []
    for alloc in nc.m.functions[0].allocations:
        if not isinstance(alloc, mybir.MemoryLocationSet):
            continue
        name = alloc.memorylocations[0].name
        if alloc.kind == "ExternalInput":
            if name != "partition_id":
                in_names.append(name)
        elif alloc.kind == "ExternalOutput":
            out_names.append(name)
            shape = tuple(alloc.tensor_shape)
            dtype = mybir.dt.np(alloc.dtype)
            out_avals.append(jax.core.ShapedArray(shape, dtype))
            zero_shapes.append((shape, dtype))
    n_params = len(in_names)
    all_names = tuple(in_names + out_names)
    donate = tuple(range(n_params, n_params + len(out_names)))
    has_pid = nc.partition_id_tensor is not None

    def _body(*args):
        operands = list(args)
        names = all_names
        if has_pid:
            operands.append(bass2jax.partition_id_tensor())
            names = all_names + ("partition_id",)
        outs = bass2jax._bass_exec_p.bind(
            *operands, out_avals=tuple(out_avals), in_names=names,
            out_names=tuple(out_names), lowering_input_output_aliases=(),
            sim_require_finite=False, sim_require_nnan=False, nc=nc)
        return tuple(outs)

    devices = jax.devices()[:8]
    mesh = Mesh(np.asarray(devices), ("core",))
    specs = (PartitionSpec("core"),) * (n_params + len(out_names))
    f = jax.jit(shard_map(_body, mesh=mesh, in_specs=specs,
                          out_specs=(PartitionSpec("core"),) * len(out_names),
                          check_rep=False),
                donate_argnums=donate, keep_unused=True)

    def run(in_maps):
        concat_in = [np.concatenate([m[n] for m in in_maps], axis=0)
                     for n in in_names]
        concat_zeros = [np.zeros((8 * s[0], *s[1:]), d) for s, d in zero_shapes]
        outs = f(*concat_in, *concat_zeros)
        res = []
        for c in range(8):
            res.append({name: np.asarray(outs[i]).reshape(8, *out_avals[i].shape)[c]
                        for i, name in enumerate(out_names)})
        return res

    _CACHE["runner"] = run
    _CACHE["nc"] = nc
    return run


def make_in_maps(k, q, v, Wk, bk, Wq, bq, Wv, bv, Wo, bo):
    in_maps = []
    for c in range(8):
        b, g = divmod(c, 2)
        gs, ge = g * G, (g + 1) * G
        bqs = (bq[gs:ge] * SCALE).reshape(NCOUT, 128).T
        in_maps.append({
            "xq": np.ascontiguousarray(q[b].T, dtype=np.float32),
            "xk": np.ascontiguousarray(k[b].T, dtype=np.float32),
            "xv": np.ascontiguousarray(v[b].T, dtype=np.float32),
            "wq": np.ascontiguousarray(Wq[gs:ge, :].T, dtype=np.float32),
            "wk": np.ascontiguousarray(Wk[gs:ge, :].T, dtype=np.float32),
            "wv": np.ascontiguousarray(Wv[gs:ge, :].T, dtype=np.float32),
            "wo": np.ascontiguousarray(Wo[:, gs:ge].T, dtype=np.float32),
            "bq": np.ascontiguousarray(bqs, dtype=np.float32),
        })
    return in_maps


def kernel(k, q, v, Wk, bk, Wq, bq, Wv, bv, Wo, bo):
    k = np.asarray(k, dtype=np.float32)
    q = np.asarray(q, dtype=np.float32)
    v = np.asarray(v, dtype=np.float32)
    Wk, bk = np.asarray(Wk, np.float32), np.asarray(bk, np.float32)
    Wq, bq = np.asarray(Wq, np.float32), np.asarray(bq, np.float32)
    Wv, bv = np.asarray(Wv, np.float32), np.asarray(bv, np.float32)
    Wo, bo = np.asarray(Wo, np.float32), np.asarray(bo, np.float32)

    in_maps = make_in_maps(k, q, v, Wk, bk, Wq, bq, Wv, bv, Wo, bo)
    run = _get_runner()
    res = run(in_maps)
    host_bias = (bo + Wo @ bv).astype(np.float32)
    out = np.empty((B, T, C), np.float32)
    for b in range(B):
        out[b] = res[2 * b]["out"] + res[2 * b + 1]["out"] + host_bias[None, :]
    return out


# revision 23
# speedup vs baseline: 1.3383x; 1.3383x over previous
"""Trainium2 Bass kernel for multi-head attention (B=4, T=2048, C=1024, H=16).

Sharding: 8 cores = (batch b in 0..3) x (head-group g in 0..1, 8 heads each).
Per core: QKV projections for its 512 dims, attention for 8 heads, partial
output projection. Host sums the two per-batch partials and adds the biases
that fold out of the device computation:
  - bk drops entirely (softmax is invariant to per-query additive constants)
  - bv folds to host:   out += Wo @ bv   (softmax rows sum to 1)
  - bo added on host
  - bq is applied on-device in the Q-projection drain (scaled by 1/sqrt(dh))

Device layout (per core):
  QT, KT [512, 2048] fp32r  (head-dim on partitions -> scores without transpose)
  V      [2048, 512] bf16   (keys on partitions -> P@V without transpose)
  scores S^T [keys, queries] in PSUM -> exp on ACT -> E bf16 in SBUF
  denominators via ones-matmul on PE (partition-dim reduction)
  P@V accumulated over key chunks; normalize by 1/d on the DVE drain
  out-projection accumulated over the 4 head-pair chunks -> DMA to DRAM
"""
import os
import numpy as np

import concourse.bass as bass
import concourse.mybir as mybir
import concourse.tile as tile
from concourse import bacc

F32 = mybir.dt.float32
F32R = mybir.dt.float32r
BF16 = mybir.dt.bfloat16
AF = mybir.ActivationFunctionType

B, T, C = 4, 2048, 1024
H, CH = 16, 64
G = 512            # dims per head-group (8 heads)
NCIN = 8           # 128-chunks of C
NCOUT = 4          # 128-chunks of G
NTB = 4            # 512-wide t blocks
NKC = 16           # 128-wide key chunks
NQB = 4            # 512-wide query blocks
SCALE = 1.0 / np.sqrt(CH)


def build_nc(debug=False):
    nc = bacc.Bacc()
    xq = nc.declare_dram_parameter("xq", [C, T], F32R, isOutput=False)
    xk = nc.declare_dram_parameter("xk", [C, T], F32R, isOutput=False)
    xv = nc.declare_dram_parameter("xv", [C, T], F32R, isOutput=False)
    wq = nc.declare_dram_parameter("wq", [C, G], F32R, isOutput=False)
    wk = nc.declare_dram_parameter("wk", [C, G], F32R, isOutput=False)
    wv = nc.declare_dram_parameter("wv", [C, G], F32R, isOutput=False)
    wo = nc.declare_dram_parameter("wo", [G, C], F32R, isOutput=False)
    bq = nc.declare_dram_parameter("bq", [128, NCOUT], F32, isOutput=False)
    out = nc.declare_dram_parameter("out", [T, C], F32, isOutput=True)
    if debug:
        qt_d = nc.declare_dram_parameter("qt_d", [NCOUT, 128, T], F32R, isOutput=True)
        kt_d = nc.declare_dram_parameter("kt_d", [NCOUT, 128, T], F32R, isOutput=True)
        v_d = nc.declare_dram_parameter("v_d", [128, NKC, 8, 65], BF16, isOutput=True)
        dr_d = nc.declare_dram_parameter("dr_d", [2, 512], F32, isOutput=True)
        e_d = nc.declare_dram_parameter("e_d", [2, 128, NKC, 512], BF16, isOutput=True)
        di_d = nc.declare_dram_parameter("di_d", [128, 512], F32, isOutput=True)
        ot_d = nc.declare_dram_parameter("ot_d", [128, NCOUT, 512], F32R, isOutput=True)

    xq_r = xq.rearrange("(c p) t -> p c t", p=128)
    xk_r = xk.rearrange("(c p) t -> p c t", p=128)
    xv_r = xv.rearrange("(c p) t -> p c t", p=128)

    with tile.TileContext(nc) as tc:
        with tc.tile_pool(name="persist", bufs=1) as persist:
            qt = [persist.tile([128, T], F32R, tag=f"qt{i}", name=f"qt{i}") for i in range(NCOUT)]
            kt = [persist.tile([128, T], F32R, tag=f"kt{i}", name=f"kt{i}") for i in range(NCOUT)]
            # V augmented with a ones column per head: PV's row 64 = softmax denom
            v_aug = persist.tile([128, NKC, 8, 65], BF16, tag="vaug")
            nc.vector.memset(v_aug[:, :, :, 64:65], 1.0)

            # ---------------- Phase A: projections ----------------
            with tc.tile_pool(name="wA", bufs=1) as wA, \
                 tc.tile_pool(name="xs", bufs=2) as xs, \
                 tc.tile_pool(name="psA", bufs=4, space="PSUM") as psA:
                wq_sb = wA.tile([128, NCIN, G], F32R, tag="wq")
                wk_sb = wA.tile([128, NCIN, G], F32R, tag="wk")
                wv_sb = wA.tile([128, NCIN, G], F32R, tag="wv")
                bq_sb = wA.tile([128, NCOUT], F32, tag="bq")
                nc.default_dma_engine.dma_start(
                    out=wk_sb, in_=wk.rearrange("(c p) g -> p c g", p=128))

                # K projection (plain), then Q (bias+scale), then V (bf16)
                for tb in range(NTB):
                    xk_t = xs.tile([128, NCIN, 512], F32R, tag="xstream")
                    nc.default_dma_engine.dma_start(
                        out=xk_t, in_=xk_r[:, :, tb * 512:(tb + 1) * 512])
                    for co in range(NCOUT):
                        ps = psA.tile([128, 512], F32, tag="psA")
                        for ci in range(NCIN):
                            nc.tensor.matmul(
                                ps,
                                wk_sb[:, ci, co * 128:(co + 1) * 128],
                                xk_t[:, ci, :],
                                start=(ci == 0), stop=(ci == NCIN - 1))
                        nc.vector.tensor_copy(
                            out=kt[co][:, tb * 512:(tb + 1) * 512], in_=ps)
                nc.default_dma_engine.dma_start(
                    out=wq_sb, in_=wq.rearrange("(c p) g -> p c g", p=128))
                nc.default_dma_engine.dma_start(out=bq_sb, in_=bq[:, :])
                for tb in range(NTB):
                    xq_t = xs.tile([128, NCIN, 512], F32R, tag="xstream")
                    nc.default_dma_engine.dma_start(
                        out=xq_t, in_=xq_r[:, :, tb * 512:(tb + 1) * 512])
                    for co in range(NCOUT):
                        ps = psA.tile([128, 512], F32, tag="psA")
                        for ci in range(NCIN):
                            nc.tensor.matmul(
                                ps,
                                wq_sb[:, ci, co * 128:(co + 1) * 128],
                                xq_t[:, ci, :],
                                start=(ci == 0), stop=(ci == NCIN - 1))
                        nc.scalar.activation(
                            qt[co][:, tb * 512:(tb + 1) * 512], ps,
                            AF.Identity,
                            bias=bq_sb[:, co:co + 1], scale=float(SCALE))
                nc.default_dma_engine.dma_start(
                    out=wv_sb, in_=wv.rearrange("(c p) g -> p c g", p=128))
                for tb in range(NTB):
                    xv_t = xs.tile([128, NCIN, 512], F32R, tag="xstream")
                    nc.default_dma_engine.dma_start(
                        out=xv_t, in_=xv_r[:, :, tb * 512:(tb + 1) * 512])
                    for sub in range(4):
                        tcix = tb * 4 + sub
                        ps = psA.tile([128, 512], F32, tag="psA")
                        for ci in range(NCIN):
                            nc.tensor.matmul(
                                ps,
                                xv_t[:, ci, sub * 128:(sub + 1) * 128],
                                wv_sb[:, ci, :],
                                start=(ci == 0), stop=(ci == NCIN - 1))
                        nc.vector.tensor_copy(out=v_aug[:, tcix, :, 0:64], in_=ps)
                if debug:
                    for co in range(NCOUT):
                        nc.default_dma_engine.dma_start(out=qt_d[co], in_=qt[co][:, :])
                        nc.default_dma_engine.dma_start(out=kt_d[co], in_=kt[co][:, :])
                    nc.default_dma_engine.dma_start(
                        out=v_d[:, :, :, :], in_=v_aug[:, :, :, :])

            # ---------------- Phase B: attention + out-proj ----------------
            with tc.tile_pool(name="wB", bufs=1) as wB, \
                 tc.tile_pool(name="eb", bufs=2) as eb, \
                 tc.tile_pool(name="otp", bufs=2) as otp, \
                 tc.tile_pool(name="dv", bufs=2) as dv, \
                 tc.tile_pool(name="scp", bufs=1, space="PSUM") as scp, \
                 tc.tile_pool(name="pvp", bufs=1, space="PSUM") as pvp, \
                 tc.tile_pool(name="pjp", bufs=2, space="PSUM") as pjp:
                wo_sb = wB.tile([128, NCOUT, C], F32R, tag="wo")
                nc.default_dma_engine.dma_start(
                    out=wo_sb, in_=wo.rearrange("(c p) g -> p c g", p=128))

                for qb in range(NQB):
                    ot_t = otp.tile([128, NCOUT, 512], F32R, tag="ot")
                    for p in range(NCOUT):
                        # E for both heads of the pair: [keys, kc, head, q]
                        e01 = eb.tile([128, NKC, 2, 512], BF16, tag="e01")
                        qsl = slice(qb * 512, (qb + 1) * 512)
                        for grp in range(NKC // 2):
                            psc = scp.tile([128, 2, 2, 512], F32, tag="sc")
                            for j in range(2):
                                kc = grp * 2 + j
                                ksl = slice(kc * 128, (kc + 1) * 128)
                                # adjacent row-packed pair (rows 0-63 / 64-127)
                                nc.tensor.matmul(
                                    psc[:, j, 0, :], kt[p][0:64, ksl],
                                    qt[p][0:64, qsl], start=True, stop=True)
                                nc.tensor.matmul(
                                    psc[:, j, 1, :], kt[p][64:128, ksl],
                                    qt[p][64:128, qsl], start=True, stop=True)
                            nc.scalar.activation(
                                e01[:, grp * 2:grp * 2 + 2, :, :], psc, AF.Exp)
                        # P @ V with ones-augmented V: row 64 = denominator
                        pv0 = pvp.tile([128, 512], F32, tag="pv0")
                        pv1 = pvp.tile([128, 512], F32, tag="pv1")
                        for kc in range(NKC):
                            nc.tensor.matmul(
                                pv0[0:65, :], v_aug[:, kc, 2 * p, :],
                                e01[:, kc, 0, :],
                                start=(kc == 0), stop=(kc == NKC - 1))
                            nc.tensor.matmul(
                                pv1[0:65, :], v_aug[:, kc, 2 * p + 1, :],
                                e01[:, kc, 1, :],
                                start=(kc == 0), stop=(kc == NKC - 1))
                        d_sb0 = dv.tile([1, 512], F32, tag="dsb0")
                        d_sb1 = dv.tile([1, 512], F32, tag="dsb1")
                        nc.vector.tensor_copy(out=d_sb0[0:1, :], in_=pv0[64:65, :])
                        nc.vector.tensor_copy(out=d_sb1[0:1, :], in_=pv1[64:65, :])
                        rec_lo = dv.tile([1, 512], F32, tag="rec_lo")
                        rec_hi = dv.tile([1, 512], F32, tag="rec_hi")
                        nc.vector.reciprocal_approx_fast(rec_lo[0:1, :],
                                                         d_sb0[0:1, :])
                        nc.vector.reciprocal_approx_fast(rec_hi[0:1, :],
                                                         d_sb1[0:1, :])
                        dbc_lo = dv.tile([64, 512], F32, tag="dbc_lo")
                        dbc_hi = dv.tile([64, 512], F32, tag="dbc_hi")
                        nc.gpsimd.partition_broadcast(dbc_lo[:, :], rec_lo[0:1, :],
                                                      channels=64)
                        nc.gpsimd.partition_broadcast(dbc_hi[:, :], rec_hi[0:1, :],
                                                      channels=64)
                        nc.vector.tensor_mul(ot_t[0:64, p, :], pv0[0:64, :],
                                             dbc_lo[:, :])
                        nc.vector.tensor_mul(ot_t[64:128, p, :], pv1[0:64, :],
                                             dbc_hi[:, :])
                        if debug and qb == 0 and p == 0:
                            dr_sb0 = dv.tile([1, 512], F32, tag="dr_sb0")
                            dr_sb1 = dv.tile([1, 512], F32, tag="dr_sb1")
                            nc.vector.tensor_copy(out=dr_sb0[0:1, :], in_=pv0[64:65, :])
                            nc.vector.tensor_copy(out=dr_sb1[0:1, :], in_=pv1[64:65, :])
                            nc.default_dma_engine.dma_start(out=dr_d[0:1, :], in_=dr_sb0[:, :])
                            nc.default_dma_engine.dma_start(out=dr_d[1:2, :], in_=dr_sb1[:, :])
                            nc.default_dma_engine.dma_start(out=e_d[0], in_=e01[:, :, 0, :])
                            nc.default_dma_engine.dma_start(out=e_d[1], in_=e01[:, :, 1, :])
                            nc.default_dma_engine.dma_start(out=di_d[0:64, :], in_=dbc_lo[:, :])
                            nc.default_dma_engine.dma_start(out=di_d[64:128, :], in_=dbc_hi[:, :])
                    if debug and qb == 0:
                        nc.default_dma_engine.dma_start(out=ot_d[:, :, :], in_=ot_t[:, :, :])
                    # output projection for this query block
                    for tcx in range(4):
                        for n in range(2):
                            pj = pjp.tile([128, 512], F32, tag="pj")
                            for p in range(NCOUT):
                                nc.tensor.matmul(
                                    pj,
                                    ot_t[:, p, tcx * 128:(tcx + 1) * 128],
                                    wo_sb[:, p, n * 512:(n + 1) * 512],
                                    start=(p == 0), stop=(p == NCOUT - 1))
                            oj = dv.tile([128, 512], F32, tag="oj")
                            nc.vector.tensor_copy(out=oj, in_=pj)
                            r0 = qb * 512 + tcx * 128
                            nc.default_dma_engine.dma_start(
                                out=out[r0:r0 + 128, n * 512:(n + 1) * 512],
                                in_=oj)
    nc.finalize()
    return nc


_CACHE = {}


def _get_runner():
    """Compile once per process; return f(in_maps) -> list of out dicts."""
    if "runner" in _CACHE:
        return _CACHE["runner"]
    import jax
    from jax.sharding import Mesh, PartitionSpec
    from jax.experimental.shard_map import shard_map
    from concourse import bass2jax

    nc = build_nc()
    bass2jax.install_neuronx_cc_hook()
    in_names, out_names, out_avals, zero_shapes = [], [], [], []
    for alloc in nc.m.functions[0].allocations:
        if not isinstance(alloc, mybir.MemoryLocationSet):
            continue
        name = alloc.memorylocations[0].name
        if alloc.kind == "ExternalInput":
            if name != "partition_id":
                in_names.append(name)
        elif alloc.kind == "ExternalOutput":
            out_names.append(name)
            shape = tuple(alloc.tensor_shape)
            dtype = mybir.dt.np(alloc.dtype)
            out_avals.append(jax.core.ShapedArray(shape, dtype))
            zero_shapes.append((shape, dtype))
    n_params = len(in_names)
    all_names = tuple(in_names + out_names)
    donate = tuple(range(n_params, n_params + len(out_names)))
    has_pid = nc.partition_id_tensor is not None

    def _body(*args):
        operands = list(args)
        names = all_names
        if has_pid:
            operands.append(bass2jax.partition_id_tensor())
            names = all_names + ("partition_id",)
        outs = bass2jax._bass_exec_p.bind(
            *operands, out_avals=tuple(out_avals), in_names=names,
            out_names=tuple(out_names), lowering_input_output_aliases=(),
            sim_require_finite=False, sim_require_nnan=False, nc=nc)
        return tuple(outs)

    devices = jax.devices()[:8]
    mesh = Mesh(np.asarray(devices), ("core",))
    specs = (PartitionSpec("core"),) * (n_params + len(out_names))
    f = jax.jit(shard_map(_body, mesh=mesh, in_specs=specs,
                          out_specs=(PartitionSpec("core"),) * len(out_names),
                          check_rep=False),
                donate_argnums=donate, keep_unused=True)

    def run(in_maps):
        concat_in = [np.concatenate([m[n] for m in in_maps], axis=0)
                     for n in in_names]
        concat_zeros = [np.zeros((8 * s[0], *s[1:]), d) for s, d in zero_shapes]
        outs = f(*concat_in, *concat_zeros)
        res = []
        for c in range(8):
            res.append({name: np.asarray(outs[i]).reshape(8, *out_avals[i].shape)[c]
                        for i, name in enumerate(out_names)})
        return res

    _CACHE["runner"] = run
    _CACHE["nc"] = nc
    return run


def make_in_maps(k, q, v, Wk, bk, Wq, bq, Wv, bv, Wo, bo):
    in_maps = []
    for c in range(8):
        b, g = divmod(c, 2)
        gs, ge = g * G, (g + 1) * G
        bqs = (bq[gs:ge] * SCALE).reshape(NCOUT, 128).T
        in_maps.append({
            "xq": np.ascontiguousarray(q[b].T, dtype=np.float32),
            "xk": np.ascontiguousarray(k[b].T, dtype=np.float32),
            "xv": np.ascontiguousarray(v[b].T, dtype=np.float32),
            "wq": np.ascontiguousarray(Wq[gs:ge, :].T, dtype=np.float32),
            "wk": np.ascontiguousarray(Wk[gs:ge, :].T, dtype=np.float32),
            "wv": np.ascontiguousarray(Wv[gs:ge, :].T, dtype=np.float32),
            "wo": np.ascontiguousarray(Wo[:, gs:ge].T, dtype=np.float32),
            "bq": np.ascontiguousarray(bqs, dtype=np.float32),
        })
    return in_maps


def kernel(k, q, v, Wk, bk, Wq, bq, Wv, bv, Wo, bo):
    k = np.asarray(k, dtype=np.float32)
    q = np.asarray(q, dtype=np.float32)
    v = np.asarray(v, dtype=np.float32)
    Wk, bk = np.asarray(Wk, np.float32), np.asarray(bk, np.float32)
    Wq, bq = np.asarray(Wq, np.float32), np.asarray(bq, np.float32)
    Wv, bv = np.asarray(Wv, np.float32), np.asarray(bv, np.float32)
    Wo, bo = np.asarray(Wo, np.float32), np.asarray(bo, np.float32)

    in_maps = make_in_maps(k, q, v, Wk, bk, Wq, bq, Wv, bv, Wo, bo)
    run = _get_runner()
    res = run(in_maps)
    host_bias = (bo + Wo @ bv).astype(np.float32)
    out = np.empty((B, T, C), np.float32)
    for b in range(B):
        out[b] = res[2 * b]["out"] + res[2 * b + 1]["out"] + host_bias[None, :]
    return out


# revision 26
# speedup vs baseline: 1.5687x; 1.1721x over previous
"""Trainium2 Bass kernel for multi-head attention (B=4, T=2048, C=1024, H=16).

Sharding: 8 cores = (batch b in 0..3) x (head-group g in 0..1, 8 heads each).
Per core: QKV projections for its 512 dims, attention for 8 heads, partial
output projection. Host sums the two per-batch partials and adds the biases
that fold out of the device computation:
  - bk drops entirely (softmax is invariant to per-query additive constants)
  - bv folds to host:   out += Wo @ bv   (softmax rows sum to 1)
  - bo added on host
  - bq is applied on-device in the Q-projection drain (scaled by 1/sqrt(dh))

Device layout (per core):
  QT, KT [512, 2048] fp32r  (head-dim on partitions -> scores without transpose)
  V      [2048, 512] bf16   (keys on partitions -> P@V without transpose)
  scores S^T [keys, queries] in PSUM -> exp on ACT -> E bf16 in SBUF
  denominators via ones-matmul on PE (partition-dim reduction)
  P@V accumulated over key chunks; normalize by 1/d on the DVE drain
  out-projection accumulated over the 4 head-pair chunks -> DMA to DRAM
"""
import os
import numpy as np

import concourse.bass as bass
import concourse.mybir as mybir
import concourse.tile as tile
from concourse import bacc

F32 = mybir.dt.float32
F32R = mybir.dt.float32r
BF16 = mybir.dt.bfloat16
AF = mybir.ActivationFunctionType

B, T, C = 4, 2048, 1024
H, CH = 16, 64
G = 512            # dims per head-group (8 heads)
NCIN = 8           # 128-chunks of C
NCOUT = 4          # 128-chunks of G
NTB = 4            # 512-wide t blocks
NKC = 16           # 128-wide key chunks
NQB = 4            # 512-wide query blocks
SCALE = 1.0 / np.sqrt(CH)


def build_nc(debug=False):
    nc = bacc.Bacc()
    xq = nc.declare_dram_parameter("xq", [C, T], F32R, isOutput=False)
    xk = nc.declare_dram_parameter("xk", [C, T], F32R, isOutput=False)
    xv = nc.declare_dram_parameter("xv", [C, T], F32R, isOutput=False)
    wq = nc.declare_dram_parameter("wq", [C, G], F32R, isOutput=False)
    wk = nc.declare_dram_parameter("wk", [C, G], F32R, isOutput=False)
    wv = nc.declare_dram_parameter("wv", [C, G], F32R, isOutput=False)
    wo = nc.declare_dram_parameter("wo", [G, C], F32R, isOutput=False)
    bq = nc.declare_dram_parameter("bq", [128, NCOUT], F32, isOutput=False)
    out = nc.declare_dram_parameter("out", [T, C], F32, isOutput=True)
    if debug:
        qt_d = nc.declare_dram_parameter("qt_d", [NCOUT, 128, T], F32R, isOutput=True)
        kt_d = nc.declare_dram_parameter("kt_d", [NCOUT, 128, T], F32R, isOutput=True)
        v_d = nc.declare_dram_parameter("v_d", [128, NKC, 8, 65], BF16, isOutput=True)
        dr_d = nc.declare_dram_parameter("dr_d", [2, 512], F32, isOutput=True)
        e_d = nc.declare_dram_parameter("e_d", [2, 128, NKC, 512], BF16, isOutput=True)
        di_d = nc.declare_dram_parameter("di_d", [128, 512], F32, isOutput=True)
        ot_d = nc.declare_dram_parameter("ot_d", [128, NCOUT, 512], F32R, isOutput=True)

    xq_r = xq.rearrange("(c p) t -> p c t", p=128)
    xk_r = xk.rearrange("(c p) t -> p c t", p=128)
    xv_r = xv.rearrange("(c p) t -> p c t", p=128)

    with tile.TileContext(nc) as tc:
        with tc.tile_pool(name="persist", bufs=1) as persist:
            qt = [persist.tile([128, T], F32R, tag=f"qt{i}", name=f"qt{i}") for i in range(NCOUT)]
            kt = [persist.tile([128, T], F32R, tag=f"kt{i}", name=f"kt{i}") for i in range(NCOUT)]
            # V augmented with a ones column per head: PV's row 64 = softmax denom
            v_aug = persist.tile([128, NKC, 8, 65], BF16, tag="vaug")
            nc.vector.memset(v_aug[:, :, :, 64:65], 1.0)

            # ---------------- Phase A: projections ----------------
            with tc.tile_pool(name="wA", bufs=1) as wA, \
                 tc.tile_pool(name="xs", bufs=2) as xs, \
                 tc.tile_pool(name="psA", bufs=4, space="PSUM") as psA:
                wq_sb = wA.tile([128, NCIN, G], F32R, tag="wq")
                wk_sb = wA.tile([128, NCIN, G], F32R, tag="wk")
                wv_sb = wA.tile([128, NCIN, G], F32R, tag="wv")
                bq_sb = wA.tile([128, NCOUT], F32, tag="bq")
                wk_r = wk.rearrange("(c p) g -> p c g", p=128)
                for ci in range(NCIN):
                    nc.default_dma_engine.dma_start(out=wk_sb[:, ci, :],
                                                    in_=wk_r[:, ci, :])

                # K projection (plain), then Q (bias+scale), then V (bf16)
                for tb in range(NTB):
                    xk_t = xs.tile([128, NCIN, 512], F32R, tag="xstream")
                    for ci in range(NCIN):
                        nc.default_dma_engine.dma_start(
                            out=xk_t[:, ci, :],
                            in_=xk_r[:, ci, tb * 512:(tb + 1) * 512])
                    for co in range(NCOUT):
                        ps = psA.tile([128, 512], F32, tag="psA")
                        for ci in range(NCIN):
                            nc.tensor.matmul(
                                ps,
                                wk_sb[:, ci, co * 128:(co + 1) * 128],
                                xk_t[:, ci, :],
                                start=(ci == 0), stop=(ci == NCIN - 1))
                        nc.vector.tensor_copy(
                            out=kt[co][:, tb * 512:(tb + 1) * 512], in_=ps)
                wq_r2 = wq.rearrange("(c p) g -> p c g", p=128)
                for ci in range(NCIN):
                    nc.default_dma_engine.dma_start(out=wq_sb[:, ci, :],
                                                    in_=wq_r2[:, ci, :])
                nc.default_dma_engine.dma_start(out=bq_sb, in_=bq[:, :])
                for tb in range(NTB):
                    xq_t = xs.tile([128, NCIN, 512], F32R, tag="xstream")
                    for ci in range(NCIN):
                        nc.default_dma_engine.dma_start(
                            out=xq_t[:, ci, :],
                            in_=xq_r[:, ci, tb * 512:(tb + 1) * 512])
                    for co in range(NCOUT):
                        ps = psA.tile([128, 512], F32, tag="psA")
                        for ci in range(NCIN):
                            nc.tensor.matmul(
                                ps,
                                wq_sb[:, ci, co * 128:(co + 1) * 128],
                                xq_t[:, ci, :],
                                start=(ci == 0), stop=(ci == NCIN - 1))
                        nc.scalar.activation(
                            qt[co][:, tb * 512:(tb + 1) * 512], ps,
                            AF.Identity,
                            bias=bq_sb[:, co:co + 1], scale=float(SCALE))
                wv_r2 = wv.rearrange("(c p) g -> p c g", p=128)
                for ci in range(NCIN):
                    nc.default_dma_engine.dma_start(out=wv_sb[:, ci, :],
                                                    in_=wv_r2[:, ci, :])
                for tb in range(NTB):
                    xv_t = xs.tile([128, NCIN, 512], F32R, tag="xstream")
                    for ci in range(NCIN):
                        nc.default_dma_engine.dma_start(
                            out=xv_t[:, ci, :],
                            in_=xv_r[:, ci, tb * 512:(tb + 1) * 512])
                    for sub in range(4):
                        tcix = tb * 4 + sub
                        ps = psA.tile([128, 512], F32, tag="psA")
                        for ci in range(NCIN):
                            nc.tensor.matmul(
                                ps,
                                xv_t[:, ci, sub * 128:(sub + 1) * 128],
                                wv_sb[:, ci, :],
                                start=(ci == 0), stop=(ci == NCIN - 1))
                        nc.vector.tensor_copy(out=v_aug[:, tcix, :, 0:64], in_=ps)
                if debug:
                    for co in range(NCOUT):
                        nc.default_dma_engine.dma_start(out=qt_d[co], in_=qt[co][:, :])
                        nc.default_dma_engine.dma_start(out=kt_d[co], in_=kt[co][:, :])
                    nc.default_dma_engine.dma_start(
                        out=v_d[:, :, :, :], in_=v_aug[:, :, :, :])

            # ---------------- Phase B: attention + out-proj ----------------
            with tc.tile_pool(name="wB", bufs=1) as wB, \
                 tc.tile_pool(name="eb", bufs=2) as eb, \
                 tc.tile_pool(name="otp", bufs=2) as otp, \
                 tc.tile_pool(name="dv", bufs=2) as dv, \
                 tc.tile_pool(name="scp", bufs=2, space="PSUM") as scp, \
                 tc.tile_pool(name="pvp", bufs=1, space="PSUM") as pvp, \
                 tc.tile_pool(name="pjp", bufs=2, space="PSUM") as pjp:
                wo_sb = wB.tile([128, NCOUT, C], F32R, tag="wo")
                nc.default_dma_engine.dma_start(
                    out=wo_sb, in_=wo.rearrange("(c p) g -> p c g", p=128))

                for qb in range(NQB):
                    ot_t = otp.tile([128, NCOUT, 512], F32R, tag="ot")
                    for p in range(NCOUT):
                        # E for both heads of the pair: [keys, kc, head, q]
                        e01 = eb.tile([128, NKC, 2, 512], BF16, tag="e01")
                        qsl = slice(qb * 512, (qb + 1) * 512)
                        for kc in range(NKC):
                            psc = scp.tile([128, 2, 512], F32, tag="sc")
                            ksl = slice(kc * 128, (kc + 1) * 128)
                            # adjacent row-packed pair (rows 0-63 / 64-127)
                            nc.tensor.matmul(
                                psc[:, 0, :], kt[p][0:64, ksl],
                                qt[p][0:64, qsl], start=True, stop=True)
                            nc.tensor.matmul(
                                psc[:, 1, :], kt[p][64:128, ksl],
                                qt[p][64:128, qsl], start=True, stop=True)
                            nc.scalar.activation(
                                e01[:, kc, :, :], psc, AF.Exp)
                        # P @ V with ones-augmented V: row 64 = denominator
                        pv0 = pvp.tile([128, 512], F32, tag="pv0")
                        pv1 = pvp.tile([128, 512], F32, tag="pv1")
                        for kc in range(NKC):
                            nc.tensor.matmul(
                                pv0[0:65, :], v_aug[:, kc, 2 * p, :],
                                e01[:, kc, 0, :],
                                start=(kc == 0), stop=(kc == NKC - 1))
                            nc.tensor.matmul(
                                pv1[0:65, :], v_aug[:, kc, 2 * p + 1, :],
                                e01[:, kc, 1, :],
                                start=(kc == 0), stop=(kc == NKC - 1))
                        d_sb0 = dv.tile([1, 512], F32, tag="dsb0")
                        d_sb1 = dv.tile([1, 512], F32, tag="dsb1")
                        nc.vector.tensor_copy(out=d_sb0[0:1, :], in_=pv0[64:65, :])
                        nc.vector.tensor_copy(out=d_sb1[0:1, :], in_=pv1[64:65, :])
                        rec_lo = dv.tile([1, 512], F32, tag="rec_lo")
                        rec_hi = dv.tile([1, 512], F32, tag="rec_hi")
                        nc.vector.reciprocal_approx_fast(rec_lo[0:1, :],
                                                         d_sb0[0:1, :])
                        nc.vector.reciprocal_approx_fast(rec_hi[0:1, :],
                                                         d_sb1[0:1, :])
                        dbc_lo = dv.tile([64, 512], F32, tag="dbc_lo")
                        dbc_hi = dv.tile([64, 512], F32, tag="dbc_hi")
                        nc.gpsimd.partition_broadcast(dbc_lo[:, :], rec_lo[0:1, :],
                                                      channels=64)
                        nc.gpsimd.partition_broadcast(dbc_hi[:, :], rec_hi[0:1, :],
                                                      channels=64)
                        nc.vector.tensor_mul(ot_t[0:64, p, :], pv0[0:64, :],
                                             dbc_lo[:, :])
                        nc.vector.tensor_mul(ot_t[64:128, p, :], pv1[0:64, :],
                                             dbc_hi[:, :])
                        if debug and qb == 0 and p == 0:
                            dr_sb0 = dv.tile([1, 512], F32, tag="dr_sb0")
                            dr_sb1 = dv.tile([1, 512], F32, tag="dr_sb1")
                            nc.vector.tensor_copy(out=dr_sb0[0:1, :], in_=pv0[64:65, :])
                            nc.vector.tensor_copy(out=dr_sb1[0:1, :], in_=pv1[64:65, :])
                            nc.default_dma_engine.dma_start(out=dr_d[0:1, :], in_=dr_sb0[:, :])
                            nc.default_dma_engine.dma_start(out=dr_d[1:2, :], in_=dr_sb1[:, :])
                            nc.default_dma_engine.dma_start(out=e_d[0], in_=e01[:, :, 0, :])
                            nc.default_dma_engine.dma_start(out=e_d[1], in_=e01[:, :, 1, :])
                            nc.default_dma_engine.dma_start(out=di_d[0:64, :], in_=dbc_lo[:, :])
                            nc.default_dma_engine.dma_start(out=di_d[64:128, :], in_=dbc_hi[:, :])
                    if debug and qb == 0:
                        nc.default_dma_engine.dma_start(out=ot_d[:, :, :], in_=ot_t[:, :, :])
                    # output projection for this query block
                    for tcx in range(4):
                        for n in range(2):
                            pj = pjp.tile([128, 512], F32, tag="pj")
                            for p in range(NCOUT):
                                nc.tensor.matmul(
                                    pj,
                                    ot_t[:, p, tcx * 128:(tcx + 1) * 128],
                                    wo_sb[:, p, n * 512:(n + 1) * 512],
                                    start=(p == 0), stop=(p == NCOUT - 1))
                            oj = dv.tile([128, 512], F32, tag="oj")
                            nc.vector.tensor_copy(out=oj, in_=pj)
                            r0 = qb * 512 + tcx * 128
                            nc.default_dma_engine.dma_start(
                                out=out[r0:r0 + 128, n * 512:(n + 1) * 512],
                                in_=oj)
    nc.finalize()
    return nc


_CACHE = {}


def _get_runner():
    """Compile once per process; return f(in_maps) -> list of out dicts."""
    if "runner" in _CACHE:
        return _CACHE["runner"]
    import jax
    from jax.sharding import Mesh, PartitionSpec
    from jax.experimental.shard_map import shard_map
    from concourse import bass2jax

    nc = build_nc()
    bass2jax.install_neuronx_cc_hook()
    in_names, out_names, out_avals, zero_shapes = [], [], [], []
    for alloc in nc.m.functions[0].allocations:
        if not isinstance(alloc, mybir.MemoryLocationSet):
            continue
        name = alloc.memorylocations[0].name
        if alloc.kind == "ExternalInput":
            if name != "partition_id":
                in_names.append(name)
        elif alloc.kind == "ExternalOutput":
            out_names.append(name)
            shape = tuple(alloc.tensor_shape)
            dtype = mybir.dt.np(alloc.dtype)
            out_avals.append(jax.core.ShapedArray(shape, dtype))
            zero_shapes.append((shape, dtype))
    n_params = len(in_names)
    all_names = tuple(in_names + out_names)
    donate = tuple(range(n_params, n_params + len(out_names)))
    has_pid = nc.partition_id_tensor is not None

    def _body(*args):
        operands = list(args)
        names = all_names
        if has_pid:
            operands.append(bass2jax.partition_id_tensor())
            names = all_names + ("partition_id",)
        outs = bass2jax._bass_exec_p.bind(
            *operands, out_avals=tuple(out_avals), in_names=names,
            out_names=tuple(out_names), lowering_input_output_aliases=(),
            sim_require_finite=False, sim_require_nnan=False, nc=nc)
        return tuple(outs)

    devices = jax.devices()[:8]
    mesh = Mesh(np.asarray(devices), ("core",))
    specs = (PartitionSpec("core"),) * (n_params + len(out_names))
    f = jax.jit(shard_map(_body, mesh=mesh, in_specs=specs,
                          out_specs=(PartitionSpec("core"),) * len(out_names),
                          check_rep=False),
                donate_argnums=donate, keep_unused=True)

    def run(in_maps):
        concat_in = [np.concatenate([m[n] for m in in_maps], axis=0)
                     for n in in_names]
        concat_zeros = [np.zeros((8 * s[0], *s[1:]), d) for s, d in zero_shapes]
        outs = f(*concat_in, *concat_zeros)
        res = []
        for c in range(8):
            res.append({name: np.asarray(outs[i]).reshape(8, *out_avals[i].shape)[c]
                        for i, name in enumerate(out_names)})
        return res

    _CACHE["runner"] = run
    _CACHE["nc"] = nc
    return run


def make_in_maps(k, q, v, Wk, bk, Wq, bq, Wv, bv, Wo, bo):
    in_maps = []
    for c in range(8):
        b, g = divmod(c, 2)
        gs, ge = g * G, (g + 1) * G
        bqs = (bq[gs:ge] * SCALE).reshape(NCOUT, 128).T
        in_maps.append({
            "xq": np.ascontiguousarray(q[b].T, dtype=np.float32),
            "xk": np.ascontiguousarray(k[b].T, dtype=np.float32),
            "xv": np.ascontiguousarray(v[b].T, dtype=np.float32),
            "wq": np.ascontiguousarray(Wq[gs:ge, :].T, dtype=np.float32),
            "wk": np.ascontiguousarray(Wk[gs:ge, :].T, dtype=np.float32),
            "wv": np.ascontiguousarray(Wv[gs:ge, :].T, dtype=np.float32),
            "wo": np.ascontiguousarray(Wo[:, gs:ge].T, dtype=np.float32),
            "bq": np.ascontiguousarray(bqs, dtype=np.float32),
        })
    return in_maps


def kernel(k, q, v, Wk, bk, Wq, bq, Wv, bv, Wo, bo):
    k = np.asarray(k, dtype=np.float32)
    q = np.asarray(q, dtype=np.float32)
    v = np.asarray(v, dtype=np.float32)
    Wk, bk = np.asarray(Wk, np.float32), np.asarray(bk, np.float32)
    Wq, bq = np.asarray(Wq, np.float32), np.asarray(bq, np.float32)
    Wv, bv = np.asarray(Wv, np.float32), np.asarray(bv, np.float32)
    Wo, bo = np.asarray(Wo, np.float32), np.asarray(bo, np.float32)

    in_maps = make_in_maps(k, q, v, Wk, bk, Wq, bq, Wv, bv, Wo, bo)
    run = _get_runner()
    res = run(in_maps)
    host_bias = (bo + Wo @ bv).astype(np.float32)
    out = np.empty((B, T, C), np.float32)
    for b in range(B):
        out[b] = res[2 * b]["out"] + res[2 * b + 1]["out"] + host_bias[None, :]
    return out


# revision 27
# speedup vs baseline: 1.6797x; 1.0708x over previous
"""Trainium2 Bass kernel for multi-head attention (B=4, T=2048, C=1024, H=16).

Sharding: 8 cores = (batch b in 0..3) x (head-group g in 0..1, 8 heads each).
Per core: QKV projections for its 512 dims, attention for 8 heads, partial
output projection. Host sums the two per-batch partials and adds the biases
that fold out of the device computation:
  - bk drops entirely (softmax is invariant to per-query additive constants)
  - bv folds to host:   out += Wo @ bv   (softmax rows sum to 1)
  - bo added on host
  - bq is applied on-device in the Q-projection drain (scaled by 1/sqrt(dh))

Device layout (per core):
  QT, KT [512, 2048] fp32r  (head-dim on partitions -> scores without transpose)
  V      [2048, 512] bf16   (keys on partitions -> P@V without transpose)
  scores S^T [keys, queries] in PSUM -> exp on ACT -> E bf16 in SBUF
  denominators via ones-matmul on PE (partition-dim reduction)
  P@V accumulated over key chunks; normalize by 1/d on the DVE drain
  out-projection accumulated over the 4 head-pair chunks -> DMA to DRAM
"""
import os
import numpy as np

import concourse.bass as bass
import concourse.mybir as mybir
import concourse.tile as tile
from concourse import bacc

F32 = mybir.dt.float32
F32R = mybir.dt.float32r
BF16 = mybir.dt.bfloat16
AF = mybir.ActivationFunctionType

B, T, C = 4, 2048, 1024
H, CH = 16, 64
G = 512            # dims per head-group (8 heads)
NCIN = 8           # 128-chunks of C
NCOUT = 4          # 128-chunks of G
NTB = 4            # 512-wide t blocks
NKC = 16           # 128-wide key chunks
NQB = 4            # 512-wide query blocks
SCALE = 1.0 / np.sqrt(CH)


def build_nc(debug=False):
    nc = bacc.Bacc()
    xq = nc.declare_dram_parameter("xq", [C, T], F32R, isOutput=False)
    xk = nc.declare_dram_parameter("xk", [C, T], F32R, isOutput=False)
    xv = nc.declare_dram_parameter("xv", [C, T], F32R, isOutput=False)
    wq = nc.declare_dram_parameter("wq", [C, G], F32R, isOutput=False)
    wk = nc.declare_dram_parameter("wk", [C, G], F32R, isOutput=False)
    wv = nc.declare_dram_parameter("wv", [C, G], F32R, isOutput=False)
    wo = nc.declare_dram_parameter("wo", [G, C], F32R, isOutput=False)
    bq = nc.declare_dram_parameter("bq", [128, NCOUT], F32, isOutput=False)
    out = nc.declare_dram_parameter("out", [T, C], F32, isOutput=True)
    if debug:
        qt_d = nc.declare_dram_parameter("qt_d", [NCOUT, 128, T], BF16, isOutput=True)
        kt_d = nc.declare_dram_parameter("kt_d", [NCOUT, 128, T], BF16, isOutput=True)
        v_d = nc.declare_dram_parameter("v_d", [128, NKC, 8, 65], BF16, isOutput=True)
        dr_d = nc.declare_dram_parameter("dr_d", [2, 512], F32, isOutput=True)
        e_d = nc.declare_dram_parameter("e_d", [2, 128, NKC, 512], BF16, isOutput=True)
        di_d = nc.declare_dram_parameter("di_d", [128, 512], F32, isOutput=True)
        ot_d = nc.declare_dram_parameter("ot_d", [128, NCOUT, 512], F32R, isOutput=True)

    xq_r = xq.rearrange("(c p) t -> p c t", p=128)
    xk_r = xk.rearrange("(c p) t -> p c t", p=128)
    xv_r = xv.rearrange("(c p) t -> p c t", p=128)

    with tile.TileContext(nc) as tc:
        with tc.tile_pool(name="persist", bufs=1) as persist:
            qt = [persist.tile([128, T], BF16, tag=f"qt{i}", name=f"qt{i}") for i in range(NCOUT)]
            kt = [persist.tile([128, T], BF16, tag=f"kt{i}", name=f"kt{i}") for i in range(NCOUT)]
            # V augmented with a ones column per head: PV's row 64 = softmax denom
            v_aug = persist.tile([128, NKC, 8, 65], BF16, tag="vaug")
            nc.vector.memset(v_aug[:, :, :, 64:65], 1.0)

            # ---------------- Phase A: projections ----------------
            with tc.tile_pool(name="wA", bufs=1) as wA, \
                 tc.tile_pool(name="xs", bufs=2) as xs, \
                 tc.tile_pool(name="psA", bufs=4, space="PSUM") as psA:
                wq_sb = wA.tile([128, NCIN, G], F32R, tag="wq")
                wk_sb = wA.tile([128, NCIN, G], F32R, tag="wk")
                wv_sb = wA.tile([128, NCIN, G], F32R, tag="wv")
                bq_sb = wA.tile([128, NCOUT], F32, tag="bq")
                wk_r = wk.rearrange("(c p) g -> p c g", p=128)
                for ci in range(NCIN):
                    nc.default_dma_engine.dma_start(out=wk_sb[:, ci, :],
                                                    in_=wk_r[:, ci, :])

                # K projection (plain), then Q (bias+scale), then V (bf16)
                for tb in range(NTB):
                    xk_t = xs.tile([128, NCIN, 512], F32R, tag="xstream")
                    for ci in range(NCIN):
                        nc.default_dma_engine.dma_start(
                            out=xk_t[:, ci, :],
                            in_=xk_r[:, ci, tb * 512:(tb + 1) * 512])
                    for co in range(NCOUT):
                        ps = psA.tile([128, 512], F32, tag="psA")
                        for ci in range(NCIN):
                            nc.tensor.matmul(
                                ps,
                                wk_sb[:, ci, co * 128:(co + 1) * 128],
                                xk_t[:, ci, :],
                                start=(ci == 0), stop=(ci == NCIN - 1))
                        nc.vector.tensor_copy(
                            out=kt[co][:, tb * 512:(tb + 1) * 512], in_=ps)
                wq_r2 = wq.rearrange("(c p) g -> p c g", p=128)
                for ci in range(NCIN):
                    nc.default_dma_engine.dma_start(out=wq_sb[:, ci, :],
                                                    in_=wq_r2[:, ci, :])
                nc.default_dma_engine.dma_start(out=bq_sb, in_=bq[:, :])
                for tb in range(NTB):
                    xq_t = xs.tile([128, NCIN, 512], F32R, tag="xstream")
                    for ci in range(NCIN):
                        nc.default_dma_engine.dma_start(
                            out=xq_t[:, ci, :],
                            in_=xq_r[:, ci, tb * 512:(tb + 1) * 512])
                    for co in range(NCOUT):
                        ps = psA.tile([128, 512], F32, tag="psA")
                        for ci in range(NCIN):
                            nc.tensor.matmul(
                                ps,
                                wq_sb[:, ci, co * 128:(co + 1) * 128],
                                xq_t[:, ci, :],
                                start=(ci == 0), stop=(ci == NCIN - 1))
                        nc.scalar.activation(
                            qt[co][:, tb * 512:(tb + 1) * 512], ps,
                            AF.Identity,
                            bias=bq_sb[:, co:co + 1], scale=float(SCALE))
                wv_r2 = wv.rearrange("(c p) g -> p c g", p=128)
                for ci in range(NCIN):
                    nc.default_dma_engine.dma_start(out=wv_sb[:, ci, :],
                                                    in_=wv_r2[:, ci, :])
                for tb in range(NTB):
                    xv_t = xs.tile([128, NCIN, 512], F32R, tag="xstream")
                    for ci in range(NCIN):
                        nc.default_dma_engine.dma_start(
                            out=xv_t[:, ci, :],
                            in_=xv_r[:, ci, tb * 512:(tb + 1) * 512])
                    for sub in range(4):
                        tcix = tb * 4 + sub
                        ps = psA.tile([128, 512], F32, tag="psA")
                        for ci in range(NCIN):
                            nc.tensor.matmul(
                                ps,
                                xv_t[:, ci, sub * 128:(sub + 1) * 128],
                                wv_sb[:, ci, :],
                                start=(ci == 0), stop=(ci == NCIN - 1))
                        nc.vector.tensor_copy(out=v_aug[:, tcix, :, 0:64], in_=ps)
                if debug:
                    for co in range(NCOUT):
                        nc.default_dma_engine.dma_start(out=qt_d[co], in_=qt[co][:, :])
                        nc.default_dma_engine.dma_start(out=kt_d[co], in_=kt[co][:, :])
                    nc.default_dma_engine.dma_start(
                        out=v_d[:, :, :, :], in_=v_aug[:, :, :, :])

            # ---------------- Phase B: attention + out-proj ----------------
            with tc.tile_pool(name="wB", bufs=1) as wB, \
                 tc.tile_pool(name="eb", bufs=2) as eb, \
                 tc.tile_pool(name="otp", bufs=2) as otp, \
                 tc.tile_pool(name="dv", bufs=2) as dv, \
                 tc.tile_pool(name="scp", bufs=2, space="PSUM") as scp, \
                 tc.tile_pool(name="pvp", bufs=1, space="PSUM") as pvp, \
                 tc.tile_pool(name="pjp", bufs=2, space="PSUM") as pjp:
                wo_sb = wB.tile([128, NCOUT, C], F32R, tag="wo")
                nc.default_dma_engine.dma_start(
                    out=wo_sb, in_=wo.rearrange("(c p) g -> p c g", p=128))

                for qb in range(NQB):
                    ot_t = otp.tile([128, NCOUT, 512], F32R, tag="ot")
                    for p in range(NCOUT):
                        # E for both heads of the pair: [keys, kc, head, q]
                        e01 = eb.tile([128, NKC, 2, 512], BF16, tag="e01")
                        qsl = slice(qb * 512, (qb + 1) * 512)
                        for kc in range(NKC):
                            psc = scp.tile([128, 2, 512], F32, tag="sc")
                            ksl = slice(kc * 128, (kc + 1) * 128)
                            # adjacent row-packed pair (rows 0-63 / 64-127)
                            nc.tensor.matmul(
                                psc[:, 0, :], kt[p][0:64, ksl],
                                qt[p][0:64, qsl], start=True, stop=True)
                            nc.tensor.matmul(
                                psc[:, 1, :], kt[p][64:128, ksl],
                                qt[p][64:128, qsl], start=True, stop=True)
                            nc.scalar.activation(
                                e01[:, kc, :, :], psc, AF.Exp)
                        # P @ V with ones-augmented V: row 64 = denominator
                        pv0 = pvp.tile([128, 512], F32, tag="pv0")
                        pv1 = pvp.tile([128, 512], F32, tag="pv1")
                        for kc in range(NKC):
                            nc.tensor.matmul(
                                pv0[0:65, :], v_aug[:, kc, 2 * p, :],
                                e01[:, kc, 0, :],
                                start=(kc == 0), stop=(kc == NKC - 1))
                            nc.tensor.matmul(
                                pv1[0:65, :], v_aug[:, kc, 2 * p + 1, :],
                                e01[:, kc, 1, :],
                                start=(kc == 0), stop=(kc == NKC - 1))
                        d_sb0 = dv.tile([1, 512], F32, tag="dsb0")
                        d_sb1 = dv.tile([1, 512], F32, tag="dsb1")
                        nc.vector.tensor_copy(out=d_sb0[0:1, :], in_=pv0[64:65, :])
                        nc.vector.tensor_copy(out=d_sb1[0:1, :], in_=pv1[64:65, :])
                        rec_lo = dv.tile([1, 512], F32, tag="rec_lo")
                        rec_hi = dv.tile([1, 512], F32, tag="rec_hi")
                        nc.vector.reciprocal_approx_fast(rec_lo[0:1, :],
                                                         d_sb0[0:1, :])
                        nc.vector.reciprocal_approx_fast(rec_hi[0:1, :],
                                                         d_sb1[0:1, :])
                        dbc_lo = dv.tile([64, 512], F32, tag="dbc_lo")
                        dbc_hi = dv.tile([64, 512], F32, tag="dbc_hi")
                        nc.gpsimd.partition_broadcast(dbc_lo[:, :], rec_lo[0:1, :],
                                                      channels=64)
                        nc.gpsimd.partition_broadcast(dbc_hi[:, :], rec_hi[0:1, :],
                                                      channels=64)
                        nc.vector.tensor_mul(ot_t[0:64, p, :], pv0[0:64, :],
                                             dbc_lo[:, :])
                        nc.vector.tensor_mul(ot_t[64:128, p, :], pv1[0:64, :],
                                             dbc_hi[:, :])
                        if debug and qb == 0 and p == 0:
                            dr_sb0 = dv.tile([1, 512], F32, tag="dr_sb0")
                            dr_sb1 = dv.tile([1, 512], F32, tag="dr_sb1")
                            nc.vector.tensor_copy(out=dr_sb0[0:1, :], in_=pv0[64:65, :])
                            nc.vector.tensor_copy(out=dr_sb1[0:1, :], in_=pv1[64:65, :])
                            nc.default_dma_engine.dma_start(out=dr_d[0:1, :], in_=dr_sb0[:, :])
                            nc.default_dma_engine.dma_start(out=dr_d[1:2, :], in_=dr_sb1[:, :])
                            nc.default_dma_engine.dma_start(out=e_d[0], in_=e01[:, :, 0, :])
                            nc.default_dma_engine.dma_start(out=e_d[1], in_=e01[:, :, 1, :])
                            nc.default_dma_engine.dma_start(out=di_d[0:64, :], in_=dbc_lo[:, :])
                            nc.default_dma_engine.dma_start(out=di_d[64:128, :], in_=dbc_hi[:, :])
                    if debug and qb == 0:
                        nc.default_dma_engine.dma_start(out=ot_d[:, :, :], in_=ot_t[:, :, :])
                    # output projection for this query block
                    for tcx in range(4):
                        for n in range(2):
                            pj = pjp.tile([128, 512], F32, tag="pj")
                            for p in range(NCOUT):
                                nc.tensor.matmul(
                                    pj,
                                    ot_t[:, p, tcx * 128:(tcx + 1) * 128],
                                    wo_sb[:, p, n * 512:(n + 1) * 512],
                                    start=(p == 0), stop=(p == NCOUT - 1))
                            oj = dv.tile([128, 512], F32, tag="oj")
                            nc.vector.tensor_copy(out=oj, in_=pj)
                            r0 = qb * 512 + tcx * 128
                            nc.default_dma_engine.dma_start(
                                out=out[r0:r0 + 128, n * 512:(n + 1) * 512],
                                in_=oj)
    nc.finalize()
    return nc


_CACHE = {}


def _get_runner():
    """Compile once per process; return f(in_maps) -> list of out dicts."""
    if "runner" in _CACHE:
        return _CACHE["runner"]
    import jax
    from jax.sharding import Mesh, PartitionSpec
    from jax.experimental.shard_map import shard_map
    from concourse import bass2jax

    nc = build_nc()
    bass2jax.install_neuronx_cc_hook()
    in_names, out_names, out_avals, zero_shapes = [], [], [], []
    for alloc in nc.m.functions[0].allocations:
        if not isinstance(alloc, mybir.MemoryLocationSet):
            continue
        name = alloc.memorylocations[0].name
        if alloc.kind == "ExternalInput":
            if name != "partition_id":
                in_names.append(name)
        elif alloc.kind == "ExternalOutput":
            out_names.append(name)
            shape = tuple(alloc.tensor_shape)
            dtype = mybir.dt.np(alloc.dtype)
            out_avals.append(jax.core.ShapedArray(shape, dtype))
            zero_shapes.append((shape, dtype))
    n_params = len(in_names)
    all_names = tuple(in_names + out_names)
    donate = tuple(range(n_params, n_params + len(out_names)))
    has_pid = nc.partition_id_tensor is not None

    def _body(*args):
        operands = list(args)
        names = all_names
        if has_pid:
            operands.append(bass2jax.partition_id_tensor())
            names = all_names + ("partition_id",)
        outs = bass2jax._bass_exec_p.bind(
            *operands, out_avals=tuple(out_avals), in_names=names,
            out_names=tuple(out_names), lowering_input_output_aliases=(),
            sim_require_finite=False, sim_require_nnan=False, nc=nc)
        return tuple(outs)

    devices = jax.devices()[:8]
    mesh = Mesh(np.asarray(devices), ("core",))
    specs = (PartitionSpec("core"),) * (n_params + len(out_names))
    f = jax.jit(shard_map(_body, mesh=mesh, in_specs=specs,
                          out_specs=(PartitionSpec("core"),) * len(out_names),
                          check_rep=False),
                donate_argnums=donate, keep_unused=True)

    def run(in_maps):
        concat_in = [np.concatenate([m[n] for m in in_maps], axis=0)
                     for n in in_names]
        concat_zeros = [np.zeros((8 * s[0], *s[1:]), d) for s, d in zero_shapes]
        outs = f(*concat_in, *concat_zeros)
        res = []
        for c in range(8):
            res.append({name: np.asarray(outs[i]).reshape(8, *out_avals[i].shape)[c]
                        for i, name in enumerate(out_names)})
        return res

    _CACHE["runner"] = run
    _CACHE["nc"] = nc
    return run


def make_in_maps(k, q, v, Wk, bk, Wq, bq, Wv, bv, Wo, bo):
    in_maps = []
    for c in range(8):
        b, g = divmod(c, 2)
        gs, ge = g * G, (g + 1) * G
        bqs = (bq[gs:ge] * SCALE).reshape(NCOUT, 128).T
        in_maps.append({
            "xq": np.ascontiguousarray(q[b].T, dtype=np.float32),
            "xk": np.ascontiguousarray(k[b].T, dtype=np.float32),
            "xv": np.ascontiguousarray(v[b].T, dtype=np.float32),
            "wq": np.ascontiguousarray(Wq[gs:ge, :].T, dtype=np.float32),
            "wk": np.ascontiguousarray(Wk[gs:ge, :].T, dtype=np.float32),
            "wv": np.ascontiguousarray(Wv[gs:ge, :].T, dtype=np.float32),
            "wo": np.ascontiguousarray(Wo[:, gs:ge].T, dtype=np.float32),
            "bq": np.ascontiguousarray(bqs, dtype=np.float32),
        })
    return in_maps


def kernel(k, q, v, Wk, bk, Wq, bq, Wv, bv, Wo, bo):
    k = np.asarray(k, dtype=np.float32)
    q = np.asarray(q, dtype=np.float32)
    v = np.asarray(v, dtype=np.float32)
    Wk, bk = np.asarray(Wk, np.float32), np.asarray(bk, np.float32)
    Wq, bq = np.asarray(Wq, np.float32), np.asarray(bq, np.float32)
    Wv, bv = np.asarray(Wv, np.float32), np.asarray(bv, np.float32)
    Wo, bo = np.asarray(Wo, np.float32), np.asarray(bo, np.float32)

    in_maps = make_in_maps(k, q, v, Wk, bk, Wq, bq, Wv, bv, Wo, bo)
    run = _get_runner()
    res = run(in_maps)
    host_bias = (bo + Wo @ bv).astype(np.float32)
    out = np.empty((B, T, C), np.float32)
    for b in range(B):
        out[b] = res[2 * b]["out"] + res[2 * b + 1]["out"] + host_bias[None, :]
    return out
